# revision 14
# baseline (speedup 1.0000x reference)
"""Trainium2 Bass kernel for nn_ContextAttention (dense_transformer).

Reference model:
  neighbor_frames [2,2,96,96] -> per-frame conv3x3(1->64)+relu -> conv3x3(64->64)+relu
  feat [B, 128, 9216], Q/K/V = 1x1 projections (64 out ch),
  attn = softmax(Q^T K / 8) [B, 9216, 9216], out = (attn @ V^T)^T -> [B, 64, 96, 96]

Distribution: 8 cores = 2 batches x 4 query-row blocks of 2304 tokens.
Each core convolves ONLY its 24-row band (+halo), projects K/V for its 2304
tokens, then AllGathers fp8 K and V^T within its 4-core batch group, so the
encoder + K/V work is sharded 4-way (the old kernel recomputed it per core).

Attention (per core: 2304 queries x 9216 keys), all fp8 DoubleRow:
  S^T[m, n] = K'[:, m] . Q'[:, n]   (fp8 e4m3, [32ch x 2sub] DoubleRow, K=64;
                                     Q',K' pre-scaled by sqrt(23.08) so the
                                     psum is 128*log2(weight)-scaled)
  exp: split between ScalarE (true exp -> e4m3, ~70% of key blocks) and DVE
       (5-op corrected-Schraudolph bit-trick -> e4m3, ~30%):
         i16 = rne(max(Sp + 896, 0))        # 128/octave log2 grid
         mnt = i16 & 0x7F                   # mantissa bits
         t   = A - B*mnt                    # quadratic interp correction
         d   = t * mnt
         i8  = rne(i16/16 - d)  -> bitcast e4m3
  AV: out[65, n] accumulated over 36 key-pair blocks, fp8 DoubleRow with
      K=256 (128 partitions x 2 subtiles); vt1 columns = [V^T + v_b | ones],
      so psum rows 0-63 = ctx + v_b*rowsum and row 64 = rowsum, making the
      final normalize out = ctx'/rowsum exactly ctx/rowsum + v_b.
k_b only shifts each query's logits by a constant -> drops out of softmax.
"""

import numpy as np
import ml_dtypes

bf16 = ml_dtypes.bfloat16
f8 = ml_dtypes.float8_e4m3fn

B = 2
NF = 2
H = W = 96
HP = 98
T = H * W            # 9216
CH = 128
NB = T // 4          # 2304 query tokens per core
NCORES = 8
WR1 = 26             # window conv1 output rows
WR2 = 24             # window conv2 output rows
NPAIR = T // 256     # 36 key pair-blocks
LPAIR = NB // 256    # 9 local pair-blocks per band
CHUNKS = [512, 512, 512, 512, 256]
SHIFT = 4.0
LOG2E = float(np.log2(np.e))
ASCALE = 128 * 0.125 * LOG2E          # 23.083: psum = ASCALE * (Q.K)
PRE = float(np.sqrt(ASCALE))          # folded into q_w and k_w on host
EXP_SCALE = 1.0 / ASCALE * 0.125      # activation: exp(psum*scale - 4)
BETA16 = 16256.0 - 128 * SHIFT * LOG2E  # bf16-domain schraudolph offset (incl -4)
QA16 = 0.3398                         # quadratic correction t = QA - QB*m
QB16 = 0.002655
DVE_PAT = 10                          # of every 10 pair blocks, first N on DVE
DVE_N = int(__import__('os').environ.get('CA_DVE_N', '3'))

_COMPILED = None
LAST_RESULTS = None


def _build_nc():
    import concourse.bass as bass
    import concourse.tile as tile
    from concourse import bacc, mybir

    f32 = mybir.dt.float32
    b16 = mybir.dt.bfloat16
    fp8 = mybir.dt.float8e4
    i16 = mybir.dt.int16
    i8 = mybir.dt.int8
    EXP = mybir.ActivationFunctionType.Exp
    RELU = mybir.ActivationFunctionType.Relu
    IDENT = mybir.ActivationFunctionType.Identity
    COPY = mybir.ActivationFunctionType.Copy
    ADD = mybir.AluOpType.add
    MAX = mybir.AluOpType.max
    AND = mybir.AluOpType.bitwise_and
    MULT = mybir.AluOpType.mult
    SUB = mybir.AluOpType.subtract

    nc = bacc.Bacc("TRN2", target_bir_lowering=False, debug=False,
                   enable_asserts=False, num_devices=NCORES)

    d_w1t = nc.dram_tensor("w1t", [64, 64], b16, kind="ExternalInput").ap()
    d_w2t = nc.dram_tensor("w2t", [CH, 9 * 64], b16, kind="ExternalInput").ap()
    d_qwt = nc.dram_tensor("qwt", [CH, 64], b16, kind="ExternalInput").ap()
    d_kwt = nc.dram_tensor("kwt", [CH, 64], b16, kind="ExternalInput").ap()
    d_vwt = nc.dram_tensor("vwt", [CH, 64], b16, kind="ExternalInput").ap()
    d_b1 = nc.dram_tensor("b1", [64, 1], f32, kind="ExternalInput").ap()
    d_b2 = nc.dram_tensor("b2", [64, 1], f32, kind="ExternalInput").ap()
    d_qb = nc.dram_tensor("qb", [64, 1], f32, kind="ExternalInput").ap()
    d_vbr = nc.dram_tensor("vbr", [CH, 4 * 64], f32, kind="ExternalInput").ap()
    d_xwin = nc.dram_tensor("xwin", [NF, 28, HP], b16, kind="ExternalInput").ap()
    d_rmsk = nc.dram_tensor("rmsk", [CH, WR1 * HP], b16, kind="ExternalInput").ap()
    d_out = nc.dram_tensor("out", [64, NB], f32, kind="ExternalOutput").ap()
    DEBUG = bool(int(__import__("os").environ.get("CA_DEBUG", "0")))
    if DEBUG:
        d_dbg_featw = nc.dram_tensor("dbg_featw", [CH, NB], b16, kind="ExternalOutput").ap()
        d_dbg_kb = nc.dram_tensor("dbg_kb", [64, NB], b16, kind="ExternalOutput").ap()
        d_dbg_vtb = nc.dram_tensor("dbg_vtb", [CH, LPAIR * 160], fp8, kind="ExternalOutput").ap()
        d_dbg_kq = nc.dram_tensor("dbg_kq", [CH, T], b16, kind="ExternalOutput").ap()
        d_dbg_vt1 = nc.dram_tensor("dbg_vt1", [CH, NPAIR * 160], fp8, kind="ExternalOutput").ap()
        d_dbg_qq = nc.dram_tensor("dbg_qq", [CH, NB], b16, kind="ExternalOutput").ap()
        d_dbg_ctxu = nc.dram_tensor("dbg_ctxu", [65, NB], f32, kind="ExternalOutput").ap()
        d_dbg_sp = nc.dram_tensor("dbg_sp", [CH, 1024], f32, kind="ExternalOutput").ap()
        d_dbg_ex = nc.dram_tensor("dbg_ex", [CH, 1024], fp8, kind="ExternalOutput").ap()

    with tile.TileContext(nc) as tc:
        _frees = []

        def _keep(pair):
            _frees.append(pair[1])
            return pair[0]

        w1t = _keep(tc.tile([64, 64], b16, name="w1t_sb"))
        w2t = _keep(tc.tile([CH, 9 * 64], b16, name="w2t_sb"))
        qwt = _keep(tc.tile([CH, 64], b16, name="qwt_sb"))
        kwt = _keep(tc.tile([CH, 64], b16, name="kwt_sb"))
        vwt = _keep(tc.tile([CH, 64], b16, name="vwt_sb"))
        b1s = _keep(tc.tile([64, 1], f32, name="b1_sb"))
        b2s = _keep(tc.tile([64, 1], f32, name="b2_sb"))
        qbs = _keep(tc.tile([64, 1], f32, name="qb_sb"))
        vbr = _keep(tc.tile([CH, 4 * 64], f32, name="vbr_sb"))
        x9w = _keep(tc.tile([64, WR1 * W], b16, name="x9w_sb"))
        r1wp = _keep(tc.tile([CH, WR1 * HP], b16, name="r1wp_sb"))
        featw = _keep(tc.tile([CH, NB], b16, name="featw_sb"))
        rmsk = _keep(tc.tile([CH, WR1 * HP], b16, name="rmsk_sb"))
        kdup = _keep(tc.tile([CH, T], b16, name="kdup_sb"))   # K on both halves
        qdup = _keep(tc.tile([CH, NB], b16, name="qdup_sb"))
        kb64 = _keep(tc.tile([64, NB], b16, name="kband_sb"))
        vtb = _keep(tc.tile([CH, LPAIR * 160], fp8, name="vtb_sb"))
        vt1 = _keep(tc.tile([CH, NPAIR * 160], fp8, name="vt1_sb"))
        ctxu = _keep(tc.tile([65, NB], f32, name="ctxu_sb"))
        rrow = _keep(tc.tile([1, NB], f32, name="recip_row"))
        ones1 = _keep(tc.tile([1, 64], f32, name="ones_sb"))
        outs = _keep(tc.tile([64, NB], f32, name="out_sb"))
        shf = _keep(tc.tile([CH, 1], f32, name="shift_sb"))

        dma = nc.sync.dma_start

        # ---- load inputs ----
        dma(w1t[:, :], d_w1t)
        dma(w2t[:, :], d_w2t)
        dma(qwt[:, :], d_qwt)
        dma(kwt[:, :], d_kwt)
        dma(vwt[:, :], d_vwt)
        dma(b1s[:, :], d_b1)
        dma(b2s[:, :], d_b2)
        dma(qbs[:, :], d_qb)
        dma(vbr[:, :], d_vbr)
        dma(rmsk[:, :], d_rmsk)
        nc.vector.memset(ones1[:, :], 1.0)
        nc.vector.memset(shf[:, :], -SHIFT)

        # ---- X9 window: 9 shifted copies of each padded band ----
        for f in range(NF):
            bp = 32 * f
            for t in range(9):
                dy, dx = t // 3, t % 3
                dma(x9w[bp + t:bp + t + 1, :].rearrange("p (h w) -> p h w", h=WR1),
                    d_xwin[f:f + 1, dy:dy + WR1, dx:dx + W])

        r1wv = r1wp[:, :].rearrange("p (h w) -> p h w", h=WR1)
        nc.vector.memset(r1wv[:, :, 0:1], 0.0)
        nc.vector.memset(r1wv[:, :, HP - 1:HP], 0.0)

        RPC = 4

        # ---- conv1 band (26 rows); relu on ScalarE; f1 staged to p64-127 ----
        w1_chunks = [(0, 4), (4, 4), (8, 4), (12, 4), (16, 4), (20, 4), (24, 2)]
        xv = x9w[:, :].rearrange("p (h w) -> p h w", h=WR1)
        dv = r1wp[:, :].rearrange("p (h w) -> p h w", h=WR1)
        with tc.tile_pool(name="psw1", bufs=4, space="PSUM") as pw1, \
             tc.tile_pool(name="stgw1", bufs=4) as sgw1:
            for (r0, nr) in w1_chunks:
                for f in range(NF):
                    bp = 32 * f
                    ps = pw1.tile([64, RPC * W], f32, tag="c1")
                    nc.tensor.matmul(
                        ps[:, 0:nr * W],
                        lhsT=w1t[bp:bp + 9, 0:64],
                        rhs=xv[bp:bp + 9, r0:r0 + nr, :],
                        start=True, stop=True,
                        tile_position=(bp, 0))
                    psv = ps[:, 0:nr * W].rearrange("p (h w) -> p h w", h=nr)
                    if f == 0:
                        nc.scalar.activation(dv[0:64, r0:r0 + nr, 1:97], psv,
                                             RELU, bias=b1s[:, :])
                    else:
                        st = sgw1.tile([64, RPC * W], b16, tag="c1s")
                        stv = st[:, 0:nr * W].rearrange("p (h w) -> p h w", h=nr)
                        nc.scalar.activation(stv, psv, RELU, bias=b1s[:, :])
                        dma(dv[64:128, r0:r0 + nr, 1:97], stv)
        nc.vector.tensor_mul(r1wp[:, :], r1wp[:, :], rmsk[:, :])

        # ---- conv2 band -> featw (24 rows) ----
        sv = r1wp[:, :].rearrange("p (h w) -> p h w", h=WR1)
        with tc.tile_pool(name="psw2", bufs=4, space="PSUM") as pw2, \
             tc.tile_pool(name="stgw2", bufs=4) as sgw2:
            for c in range(WR2 // RPC):
                pss = []
                for f in range(NF):
                    hb = 64 * f
                    ps = pw2.tile([64, RPC * W], f32, tag="c2")
                    for t in range(9):
                        dy, dx = t // 3, t % 3
                        nc.tensor.matmul(
                            ps[:, :],
                            lhsT=w2t[hb:hb + 64, bass.ts(t, 64)],
                            rhs=sv[hb:hb + 64, dy + c * RPC:dy + (c + 1) * RPC,
                                   dx:dx + W],
                            start=(t == 0), stop=(t == 8),
                            tile_position=(hb, 0))
                    pss.append(ps)
                nc.scalar.activation(featw[0:64, bass.ts(c, RPC * W)], pss[0][:, :],
                                     RELU, bias=b2s[:, :])
                st = sgw2.tile([64, RPC * W], b16, tag="c2s")
                nc.scalar.activation(st[:, :], pss[1][:, :], RELU, bias=b2s[:, :])
                dma(featw[64:128, bass.ts(c, RPC * W)], st[:, :])

        # ---- band projections: K (fp8, [32 x 2sub]) and V^T(+v_b) (fp8) ----
        QCH = [(0, 512), (512, 512), (1024, 512), (1536, 512), (2048, 256)]
        with tc.tile_pool(name="psp", bufs=4, space="PSUM") as pp:
            # K band -> kb64 [64, n] bf16
            for (q0, qn) in QCH:
                ps = pp.tile([64, 512], f32, tag="proj")
                nc.tensor.matmul(ps[:, 0:qn], lhsT=kwt[:, :],
                                 rhs=featw[:, bass.ds(q0, qn)],
                                 start=True, stop=True)
                nc.vector.tensor_copy(kb64[:, bass.ds(q0, qn)], ps[:, 0:qn])
            # V^T band: 4 m-blocks (128 tok) per psum bank, +v_b, -> fp8
            for g in range(5):                   # 18 blocks: 4+4+4+4+2
                nblk = min(4, 2 * LPAIR - 4 * g)
                ps = pp.tile([CH, 256], f32, tag="vt")
                for t in range(nblk):
                    m = 4 * g + t
                    nc.tensor.matmul(ps[:, bass.ts(t, 64)],
                                     lhsT=featw[:, bass.ds(128 * m, 128)],
                                     rhs=vwt[:, :], start=True, stop=True)
                # vtb view [p, blk, 80]: write cols 0:64 of each block
                dstv = vtb[:, bass.ds(g * 4 * 80, nblk * 80)].rearrange(
                    "p (t c) -> p t c", c=80)
                nc.vector.tensor_add(
                    dstv[:, :, 0:64],
                    ps[:, 0:nblk * 64].rearrange("p (t c) -> p t c", c=64),
                    vbr[:, 0:nblk * 64].rearrange("p (t c) -> p t c", c=64))
            nc.vector.memset(
                vtb[:, :].rearrange("p (t c) -> p t c", c=80)[:, :, 64:80], 0.0)
            nc.vector.memset(
                vtb[:, :].rearrange("p (t c) -> p t c", c=80)[:, :, 64:65], 1.0)

        # ---- AllGather K and V^T within the 4-core batch group ----
        groups = [[0, 1, 2, 3], [4, 5, 6, 7]]
        with tc.tile_pool(name="dram", bufs=1, space="DRAM") as dram:
            kb_in = dram.tile([64, NB], b16)
            kb_out = dram.tile([4, 64, NB], b16)
            vt_in = dram.tile([CH, LPAIR * 160], fp8)
            vt_out = dram.tile([4, CH, LPAIR * 160], fp8)
            nc.gpsimd.dma_start(kb_in[:, :], kb64[:, :])
            nc.gpsimd.dma_start(vt_in[:, :], vtb[:, :])
            nc.gpsimd.collective_compute(
                "AllGather", mybir.AluOpType.bypass, replica_groups=groups,
                ins=[kb_in[:, :].opt()], outs=[kb_out[:, :, :].opt()])
            nc.gpsimd.collective_compute(
                "AllGather", mybir.AluOpType.bypass, replica_groups=groups,
                ins=[vt_in[:, :].opt()], outs=[vt_out[:, :, :].opt()])

            # ---- Q band projection (overlaps the collectives) ----
            with tc.tile_pool(name="psq", bufs=4, space="PSUM") as pq:
                for (q0, qn) in QCH:
                    ps = pq.tile([64, 512], f32, tag="qproj")
                    nc.tensor.matmul(ps[:, 0:qn], lhsT=qwt[:, :],
                                     rhs=featw[:, bass.ds(q0, qn)],
                                     start=True, stop=True)
                    nc.scalar.activation(qdup[0:64, bass.ds(q0, qn)],
                                         ps[:, 0:qn], IDENT, bias=qbs[:, :])
                    dma(qdup[64:128, bass.ds(q0, qn)],
                        qdup[0:64, bass.ds(q0, qn)])

            # ---- reassemble gathered K (both halves) and vt1 ----
            nc.sync.dma_start(
                kdup[0:64, :].rearrange("p (q j) -> p q j", q=4),
                kb_out[:, :, :].rearrange("q p j -> p q j"))
            nc.sync.dma_start(
                kdup[64:128, :].rearrange("p (q j) -> p q j", q=4),
                kb_out[:, :, :].rearrange("q p j -> p q j"))
            nc.sync.dma_start(
                vt1[:, :].rearrange("p (q c) -> p q c", q=4),
                vt_out[:, :, :].rearrange("q p c -> p q c"))

        # ---- attention: bf16 S (row-packed), dual-path exp, fp8-DR AV ----
        with tc.tile_pool(name="spsum", bufs=3, space="PSUM") as sp_pool, \
             tc.tile_pool(name="expp", bufs=3) as e_pool, \
             tc.tile_pool(name="dvet", bufs=2) as t_pool, \
             tc.tile_pool(name="ctxp", bufs=2, space="PSUM") as c_pool:
            n_off = 0
            for ci, nch in enumerate(CHUNKS):
                ctx_ps = c_pool.tile([80, 512], f32, tag="ctx")
                for P in range(NPAIR):
                    sp = sp_pool.tile([128, 1024], f32, tag="sp")
                    for s in range(2):
                        hb = 64 * s
                        nc.tensor.matmul(
                            sp[:, s * 512:s * 512 + nch],
                            lhsT=kdup[hb:hb + 64, bass.ds(256 * P + 128 * s, 128)],
                            rhs=qdup[hb:hb + 64, bass.ds(n_off, nch)],
                            start=True, stop=True,
                            tile_position=(hb, 0))
                    if nch == 512:
                        spv, exv = sp[:, :], None
                    else:
                        spv = sp[:, :].rearrange("p (s x) -> p s x", x=512)[:, :, 0:nch]
                    ex = e_pool.tile([128, 1024], fp8, tag="ex")
                    exv = ex[:, :] if nch == 512 else \
                        ex[:, :].rearrange("p (s x) -> p s x", x=512)[:, :, 0:nch]
                    if P % DVE_PAT < DVE_N:
                        ti = t_pool.tile([128, 1024], i16, tag="i16")
                        tm = t_pool.tile([128, 1024], i16, tag="m16")
                        tt = t_pool.tile([128, 1024], b16, tag="tb")
                        td = t_pool.tile([128, 1024], b16, tag="td")
                        t5 = t_pool.tile([128, 1024], i16, tag="t5")

                        def _v(tile_):
                            if nch == 512:
                                return tile_[:, :]
                            return tile_[:, :].rearrange(
                                "p (s x) -> p s x", x=512)[:, :, 0:nch]
                        tiv, tmv, ttv, tdv, t5v = map(_v, (ti, tm, tt, td, t5))
                        nc.vector.tensor_scalar(tiv, spv, BETA16, 0.0, op0=ADD, op1=MAX)
                        nc.vector.tensor_scalar(tmv, tiv, 127, None, op0=AND)
                        nc.vector.tensor_scalar(ttv, tmv, -QB16, QA16, op0=MULT, op1=ADD)
                        nc.vector.tensor_tensor(tdv, ttv, tmv, op=MULT)
                        nc.vector.tensor_tensor(t5v, tiv, tdv, op=SUB)
                        nc.vector.tensor_copy(exv, t5v.bitcast(b16))
                    else:
                        nc.scalar.activation(exv, spv, EXP, bias=shf[:, :],
                                             scale=EXP_SCALE)
                    nc.tensor.matmul(
                        ctx_ps[:, 0:nch],
                        lhsT=vt1[:, bass.ds(P * 160, 160)].rearrange(
                            "p (s c) -> p s c", s=2),
                        rhs=ex[:, :].rearrange("p (s n) -> p s n", s=2)[:, :, 0:nch],
                        start=(P == 0), stop=(P == NPAIR - 1),
                        perf_mode=mybir.MatmulPerfMode.DoubleRow)
                nc.vector.tensor_copy(ctxu[:, bass.ds(n_off, nch)], ctx_ps[0:65, 0:nch])
                nc.vector.reciprocal(rrow[:, bass.ds(n_off, nch)],
                                     ctxu[64:65, bass.ds(n_off, nch)])
                n_off += nch

        # ---- normalize: out = ctx' / rowsum  (v_b already folded in) ----
        with tc.tile_pool(name="bps", bufs=2, space="PSUM") as bp_pool:
            n_off = 0
            for ci, nch in enumerate(CHUNKS):
                bps = bp_pool.tile([64, 512], f32, tag="bps")
                nc.tensor.matmul(bps[:, 0:nch], lhsT=ones1[:, :],
                                 rhs=rrow[:, bass.ds(n_off, nch)],
                                 start=True, stop=True)
                nc.vector.tensor_mul(outs[:, bass.ds(n_off, nch)],
                                     ctxu[0:64, bass.ds(n_off, nch)], bps[:, 0:nch])
                n_off += nch
        dma(d_out, outs[:, :])
        if DEBUG:
            dma(d_dbg_featw, featw[:, :])
            dma(d_dbg_kb, kb64[:, :])
            dma(d_dbg_vtb, vtb[:, :])
            dma(d_dbg_kq, kdup[:, :])
            dma(d_dbg_vt1, vt1[:, :])
            dma(d_dbg_qq, qdup[:, :])
            dma(d_dbg_ctxu, ctxu[:, :])

        for _f in reversed(_frees):
            _f()

    nc.compile()
    return nc


def _prep_in_maps(inputs):
    x = np.asarray(inputs["neighbor_frames"], np.float32)
    w1 = np.asarray(inputs["enc_w1"], np.float32)
    w2 = np.asarray(inputs["enc_w2"], np.float32)
    qw = np.asarray(inputs["q_w"], np.float32)
    kw = np.asarray(inputs["k_w"], np.float32)
    vw = np.asarray(inputs["v_w"], np.float32)
    b1 = np.asarray(inputs["enc_b1"], np.float32)
    b2 = np.asarray(inputs["enc_b2"], np.float32)
    qb = np.asarray(inputs["q_b"], np.float32)
    vb = np.asarray(inputs["v_b"], np.float32)
    # k_b intentionally unused: softmax cancels it exactly.

    # per-core query windows: input rows r0-2 .. r0+25 (zero outside image)
    xbig = np.zeros((B, NF, 102, HP), np.float32)   # row i = input row i-3
    xbig[:, :, 3:99, 1:97] = x
    xwin = np.zeros((B, 4, NF, 28, HP), np.float32)
    for q in range(4):
        r0 = q * 24
        xwin[:, q] = xbig[:, :, r0 + 1:r0 + 29, :]
    xwin = xwin.astype(bf16)

    # conv1w halo-row mask (window conv1 rows are global rows r0-1 .. r0+24)
    rmasks = []
    for q in range(4):
        m = np.ones((CH, WR1, HP), np.float32)
        if q == 0:
            m[:, 0, :] = 0.0
        if q == 3:
            m[:, WR1 - 1, :] = 0.0
        rmasks.append(np.ascontiguousarray(m.reshape(CH, WR1 * HP)).astype(bf16))

    # conv1 weights, tap-major, duplicated at partition rows 0-8 and 32-40
    w1t = np.zeros((64, 64), np.float32)
    taps = w1.reshape(64, 9).T
    w1t[0:9, :] = taps
    w1t[32:41, :] = taps
    w1t = w1t.astype(bf16)

    # conv2 weights w2t[cin, tap*64+cout], duplicated on both partition halves
    w2half = np.ascontiguousarray(
        w2.transpose(2, 3, 1, 0).reshape(9, 64, 64).transpose(1, 0, 2).reshape(64, 9 * 64))
    w2t = np.concatenate([w2half, w2half], axis=0).astype(bf16)

    # feat block layout row i = frame*64 + c  <->  reference channel c*2+frame
    perm = np.array([(i % 64) * 2 + i // 64 for i in range(CH)])
    qwt = np.ascontiguousarray((qw[:, perm] * PRE).T).astype(bf16)
    kwt = np.ascontiguousarray((kw[:, perm] * PRE).T).astype(bf16)
    vwt = np.ascontiguousarray(vw[:, perm].T).astype(bf16)

    b1c = np.ascontiguousarray(b1.reshape(64, 1))
    b2c = np.ascontiguousarray(b2.reshape(64, 1))
    qbc = np.ascontiguousarray((qb * PRE).reshape(64, 1))
    vbr = np.ascontiguousarray(np.tile(vb.reshape(1, 64), (CH, 4)).astype(np.float32))

    in_maps = []
    for core in range(NCORES):
        b = core // 4
        q = core % 4
        in_maps.append({
            "xwin": np.ascontiguousarray(xwin[b, q]),
            "rmsk": rmasks[q],
            "w1t": w1t, "w2t": w2t, "qwt": qwt, "kwt": kwt, "vwt": vwt,
            "b1": b1c, "b2": b2c, "qb": qbc, "vbr": vbr,
        })
    return in_maps


def _install_ntff_shim():
    """Provide antenv.axon_hooks (absent in this image) so
    run_bass_kernel_spmd(trace=True) can capture NTFF profiles through
    libaxon_pjrt's C ABI, and neuter the S3 artifact upload."""
    import sys, types, ctypes, contextlib

    if "antenv.axon_hooks" not in sys.modules:
        mod = types.ModuleType("antenv.axon_hooks")
        mod._hook = None
        mod.set_axon_ntff_profile_hook = lambda h: setattr(mod, "_hook", h)
        mod.get_axon_ntff_profile_hook = lambda: mod._hook
        sys.modules["antenv.axon_hooks"] = mod

        lib = ctypes.CDLL("/opt/axon/libaxon_pjrt.so")
        if hasattr(lib, "axon_start_nrt_profile"):
            lib.axon_start_nrt_profile.argtypes = [
                ctypes.POINTER(ctypes.c_int64), ctypes.c_size_t]
            lib.axon_start_nrt_profile.restype = ctypes.c_int64
            lib.axon_stop_nrt_profile.argtypes = [ctypes.c_char_p]
            lib.axon_stop_nrt_profile.restype = ctypes.c_int64

            @contextlib.contextmanager
            def _hook(output_dir, device_ids):
                import jax
                jax.devices()
                if device_ids:
                    ids = (ctypes.c_int64 * len(device_ids))(*device_ids)
                    rc = lib.axon_start_nrt_profile(ids, len(device_ids))
                else:
                    rc = lib.axon_start_nrt_profile(None, 0)
                if rc != 0:
                    raise RuntimeError(f"axon_start_nrt_profile rc={rc}")
                try:
                    yield
                finally:
                    n = lib.axon_stop_nrt_profile(str(output_dir).encode())
                    print(f"ntff profile: {n} file(s) -> {output_dir}")

            mod.set_axon_ntff_profile_hook(_hook)

    import concourse.bass_utils as _bu
    _bu.upload_artifacts = lambda tmpdir: tmpdir


def kernel(**inputs):
    global _COMPILED, LAST_RESULTS
    from concourse.bass_utils import run_bass_kernel_spmd

    if _COMPILED is None:
        _COMPILED = _build_nc()
    nc = _COMPILED

    in_maps = _prep_in_maps(inputs)
    trace = bool(int(__import__("os").environ.get("CA_TRACE", "0")))
    if trace:
        _install_ntff_shim()
    res = run_bass_kernel_spmd(nc, in_maps, core_ids=list(range(NCORES)),
                               trace=trace)
    LAST_RESULTS = res

    out = np.zeros((B, 64, T), np.float32)
    for core in range(NCORES):
        b = core // 4
        q = core % 4
        out[b, :, q * NB:(q + 1) * NB] = res.results[core]["out"]
    return out.reshape(B, 64, H, W)
